# revision 7
# baseline (speedup 1.0000x reference)
"""Gated multi-head attention on 8 TRN2 NeuronCores.

Problem: B=4, T=2048, DM=1024, H=16, D=64.
    q = split_heads(query @ Wq + bq); k, v likewise
    S = q k^T / sqrt(D); P = softmax(S); y = P v
    gate = sigmoid(einsum('bhtd,hde->bhte', y, gate_w) + gate_b)
    out = merge_heads(y * gate) @ Wo + bo

Sharding: core c = 2*b + hg handles batch b and head-group hg (8 heads,
512 of the 1024 model dims).  QKV projections are column-parallel,
out-proj is row-parallel; the host sums each batch's two partial
outputs (a 2-way reduce in the unshard step).

On-chip layout: everything stays feature-on-partition ("transposed"):
  - host supplies x^T (DM, T) in bf16, so projections need no on-chip
    transposes: q^T/k^T come out as (o, T), v as (T, o).
  - S^T tiles (k-part, q-free) are computed directly by swapping the
    matmul operands; exp is applied with the 1/sqrt(D) folded into the
    ACT scale; no max-subtraction (scores have std ~0.4, |S|<4, so exp
    is well-conditioned and softmax ratios are unchanged).
  - the AV matmul uses v as the stationary operand with an extra ones
    column, producing y'^T = (P^T)^T-contracted rows PLUS the softmax
    denominator l as one extra partition row, all unnormalized.
  - gating uses sigmoid((y/l)@gw + gb) = sigmoid(r*(y@gw + l*gb)) with
    r = 1/l applied as a per-partition ACT scale after a (t,e)-layout
    gate matmul whose contraction rows are [y rows, l row] against
    [gate_w rows, gate_b row].
  - the only transposes anywhere are tiny PE transposes: 16 per head
    for the (t,64) gate tile and 16 per head for the l strip.
"""

import numpy as np
import ml_dtypes

import concourse.bass as bass
import concourse.mybir as mybir
import concourse.tile as tile
from concourse.bass_utils import run_bass_kernel_spmd
from concourse.masks import make_identity

BF16 = mybir.dt.bfloat16
F32 = mybir.dt.float32
AF = mybir.ActivationFunctionType

B, T, DM, H, D = 4, 2048, 1024, 16, 64
HL = H // 2          # heads per core
OS = HL * D          # output shard dim (512)
NT = T // 128        # 16 t-tiles (also k-tiles)
KDM = DM // 128      # 8 contraction tiles for projections
SCALE = 1.0 / float(np.sqrt(D))
NQC = 2              # q-chunks of 1024 in the attention inner loop
QC = T // NQC


def _split_waits(nc, max_waits=1):
    """walrus in this container rejects >1 sync-wait per instruction;
    split extras onto same-engine NOPs placed immediately before."""
    n = 0
    for bb in nc.m.functions[0].blocks:
        out, changed = [], False
        for ins in bb.instructions:
            si = ins.sync_info
            if si is not None and len(si.on_wait) > max_waits:
                waits = list(si.on_wait)
                extra, keep = waits[:-max_waits], waits[-max_waits:]
                while extra:
                    chunk, extra = extra[:max_waits], extra[max_waits:]
                    out.append(mybir.InstNoOp(
                        name=f"{ins.name}-ws{n}", engine=ins.engine,
                        ins=[], outs=[],
                        sync_info=mybir.SyncInfo(on_wait=chunk, on_update=[]),
                    ))
                    n += 1
                ins.sync_info = mybir.SyncInfo(on_wait=keep,
                                               on_update=list(si.on_update))
                changed = True
            out.append(ins)
        if changed:
            bb.instructions = out
    return n


def build_nc(split_waits=True):
    nc = bass.Bass()

    xq = nc.declare_dram_parameter("xqT", [DM, T], BF16, isOutput=False)
    xk = nc.declare_dram_parameter("xkT", [DM, T], BF16, isOutput=False)
    xv = nc.declare_dram_parameter("xvT", [DM, T], BF16, isOutput=False)
    wq = nc.declare_dram_parameter("wq", [DM, OS], BF16, isOutput=False)
    wk = nc.declare_dram_parameter("wk", [DM, OS], BF16, isOutput=False)
    wv = nc.declare_dram_parameter("wv", [DM, OS], BF16, isOutput=False)
    bq = nc.declare_dram_parameter("bq", [128, OS // 128], F32, isOutput=False)
    bk = nc.declare_dram_parameter("bk", [128, OS // 128], F32, isOutput=False)
    bv = nc.declare_dram_parameter("bv", [1, OS], BF16, isOutput=False)
    wo = nc.declare_dram_parameter("wo", [OS, DM], BF16, isOutput=False)
    bo = nc.declare_dram_parameter("bo", [1, DM], BF16, isOutput=False)
    wg = nc.declare_dram_parameter("wg", [D + 1, HL, D], F32, isOutput=False)
    out = nc.declare_dram_parameter("out", [T, DM], F32, isOutput=True)

    with tile.TileContext(nc) as tc, \
         tc.tile_pool(name="singles", bufs=1) as singles:

        # ---- resident weights/constants -------------------------------
        wq_sb = singles.tile([128, KDM, OS], BF16, tag="wq")
        nc.sync.dma_start(out=wq_sb, in_=wq.rearrange("(a p) o -> p a o", p=128))
        wk_sb = singles.tile([128, KDM, OS], BF16, tag="wk")
        nc.sync.dma_start(out=wk_sb, in_=wk.rearrange("(a p) o -> p a o", p=128))
        wv_sb = singles.tile([128, KDM, OS], BF16, tag="wv")
        nc.sync.dma_start(out=wv_sb, in_=wv.rearrange("(a p) o -> p a o", p=128))
        wo_sb = singles.tile([128, OS // 128, DM], BF16, tag="wo")
        nc.sync.dma_start(out=wo_sb, in_=wo.rearrange("(a p) o -> p a o", p=128))
        wg_sb = singles.tile([128, HL, D], F32, tag="wg")
        nc.sync.dma_start(out=wg_sb[0:D + 1], in_=wg[:])
        bq_sb = singles.tile([128, OS // 128], F32, tag="bq")
        nc.sync.dma_start(out=bq_sb, in_=bq[:])
        bk_sb = singles.tile([128, OS // 128], F32, tag="bk")
        nc.sync.dma_start(out=bk_sb, in_=bk[:])
        bv_sb = singles.tile([128, OS], BF16, tag="bv")
        nc.sync.dma_start(out=bv_sb[0:1], in_=bv[:])
        bo_sb = singles.tile([128, DM], BF16, tag="bo")
        nc.sync.dma_start(out=bo_sb[0:1], in_=bo[:])
        ones_sb = singles.tile([128, 128], BF16, tag="ones")
        nc.vector.memset(ones_sb[0:1], 1.0)
        ident = singles.tile([128, 128], F32, tag="ident")
        make_identity(nc, ident)

        # ---- resident activations ------------------------------------
        qT_sb = singles.tile([128, OS // 128, T], BF16, tag="qT")
        kT_sb = singles.tile([128, OS // 128, T], BF16, tag="kT")
        vaug_sb = singles.tile([128, NT, HL, D + 1], BF16, tag="vaug")
        ygated_sb = singles.tile([128, OS // 128, T], BF16, tag="ygated")
        r_sb = singles.tile([128, HL, NT], F32, tag="r")

        # ones column of v_aug (gives the softmax denominator row)
        nc.vector.memset(vaug_sb[:, :, :, D:D + 1], 1.0)

        # ================= phase 1: projections ========================
        with tc.tile_pool(name="xT", bufs=2) as xpool, \
             tc.tile_pool(name="proj_psum", bufs=4, space="PSUM") as pp:

            def load_xT(dram):
                t = xpool.tile([128, KDM, T], BF16, tag="xT")
                for a in range(KDM):
                    nc.sync.dma_start(out=t[:, a, :],
                                      in_=dram[a * 128:(a + 1) * 128, :])
                return t

            # q and k projections: out (o-part, t-free), bias per-partition
            xq_t = load_xT(xq)
            for w_sb, x_t, b_sb, dst in ((wq_sb, xq_t, bq_sb, qT_sb),):
                for m in range(OS // 128):
                    for tch in range(T // 512):
                        ps = pp.tile([128, 512], F32, tag="pp")
                        for kt in range(KDM):
                            nc.tensor.matmul(
                                ps,
                                lhsT=w_sb[:, kt, m * 128:(m + 1) * 128],
                                rhs=x_t[:, kt, tch * 512:(tch + 1) * 512],
                                start=(kt == 0), stop=(kt == KDM - 1))
                        nc.scalar.activation(
                            out=dst[:, m, tch * 512:(tch + 1) * 512], in_=ps,
                            func=AF.Identity, bias=b_sb[:, m:m + 1], scale=1.0)

            xk_t = load_xT(xk)
            for m in range(OS // 128):
                for tch in range(T // 512):
                    ps = pp.tile([128, 512], F32, tag="pp")
                    for kt in range(KDM):
                        nc.tensor.matmul(
                            ps,
                            lhsT=wk_sb[:, kt, m * 128:(m + 1) * 128],
                            rhs=xk_t[:, kt, tch * 512:(tch + 1) * 512],
                            start=(kt == 0), stop=(kt == KDM - 1))
                    nc.scalar.activation(
                        out=kT_sb[:, m, tch * 512:(tch + 1) * 512], in_=ps,
                        func=AF.Identity, bias=bk_sb[:, m:m + 1], scale=1.0)

            # v projection: out (t-part, o-free); bias via ones-row matmul
            xv_t = load_xT(xv)
            for tt in range(NT):
                ps = pp.tile([128, OS], F32, tag="pp")
                for kt in range(KDM):
                    nc.tensor.matmul(
                        ps,
                        lhsT=xv_t[:, kt, tt * 128:(tt + 1) * 128],
                        rhs=wv_sb[:, kt, :],
                        start=(kt == 0), stop=False)
                nc.tensor.matmul(ps, lhsT=ones_sb[0:1], rhs=bv_sb[0:1],
                                 start=False, stop=True)
                nc.vector.tensor_copy(
                    out=vaug_sb[:, tt, :, 0:D],
                    in_=ps.rearrange("p (h d) -> p h d", h=HL))

        # ================= phase 2: attention + gating =================
        with tc.tile_pool(name="st_psum", bufs=2, space="PSUM") as stp, \
             tc.tile_pool(name="yaug_psum", bufs=1, space="PSUM") as yp, \
             tc.tile_pool(name="small_psum", bufs=2, space="PSUM") as smp, \
             tc.tile_pool(name="pt", bufs=2) as ptp, \
             tc.tile_pool(name="yaugT", bufs=2) as ytp, \
             tc.tile_pool(name="gate", bufs=2) as gp:

            for hl in range(HL):
                even = hl % 2 == 0
                ph, po = hl // 2, 64 * (hl % 2)
                yaugT = ytp.tile([128, T], F32, tag="yaugT")

                for qc in range(NQC):
                    ya = yp.tile([128, QC], F32, tag="yaug")
                    for kt in range(NT):
                        st = stp.tile([128, QC], F32, tag="st")
                        for nch in range(QC // 512):
                            nc.tensor.matmul(
                                st[:, nch * 512:(nch + 1) * 512],
                                lhsT=kT_sb[po:po + D, ph,
                                           kt * 128:(kt + 1) * 128],
                                rhs=qT_sb[po:po + D, ph,
                                          qc * QC + nch * 512:
                                          qc * QC + (nch + 1) * 512],
                                start=True, stop=True)
                        pt = ptp.tile([128, QC], BF16, tag="pt")
                        nc.scalar.activation(out=pt, in_=st, func=AF.Exp,
                                             scale=SCALE)
                        for nch in range(QC // 512):
                            nc.tensor.matmul(
                                ya[0:D + 1, nch * 512:(nch + 1) * 512],
                                lhsT=vaug_sb[:, kt, hl, :],
                                rhs=pt[:, nch * 512:(nch + 1) * 512],
                                start=(kt == 0), stop=(kt == NT - 1))
                    nc.vector.tensor_copy(
                        out=yaugT[0:D + 1, qc * QC:(qc + 1) * QC],
                        in_=ya[0:D + 1, :])

                # r = 1/l via 16 strip transposes
                rt = smp.tile([128, NT], F32, tag="sm")
                for i in range(NT):
                    nc.tensor.transpose(
                        rt[:, i:i + 1],
                        yaugT[D:D + 1, i * 128:(i + 1) * 128],
                        ident[D:D + 1, D:D + 1])
                nc.vector.reciprocal(out=r_sb[:, hl, :], in_=rt)

                # gate: (t,e) matmul -> sigmoid(scale=r) -> *r -> transpose
                ystage = None if even else gp.tile([128, T], BF16,
                                                    tag="ystage")
                for tt in range(NT):
                    gps = smp.tile([128, D], F32, tag="sm")
                    nc.tensor.matmul(
                        gps,
                        lhsT=yaugT[0:D + 1, tt * 128:(tt + 1) * 128],
                        rhs=wg_sb[0:D + 1, hl, :],
                        start=True, stop=True)
                    g1 = gp.tile([128, D], F32, tag="g1")
                    nc.scalar.activation(out=g1, in_=gps, func=AF.Sigmoid,
                                         scale=r_sb[:, hl, tt:tt + 1])
                    g2 = gp.tile([128, D], F32, tag="g2")
                    nc.vector.tensor_scalar_mul(g2, g1,
                                                r_sb[:, hl, tt:tt + 1])
                    gt = smp.tile([128, 128], F32, tag="sm")
                    nc.tensor.transpose(gt[0:D], g2, ident)
                    dst = (ygated_sb[0:D, ph, tt * 128:(tt + 1) * 128]
                           if even else
                           ystage[0:D, tt * 128:(tt + 1) * 128])
                    nc.vector.tensor_mul(
                        out=dst,
                        in0=yaugT[0:D, tt * 128:(tt + 1) * 128],
                        in1=gt[0:D])
                if not even:
                    # DVE cannot shift partitions; DMA the odd head's gated
                    # rows into the upper half of the pair tile.
                    nc.sync.dma_start(out=ygated_sb[64:128, ph, :],
                                      in_=ystage[0:D, :])

        # ================= phase 3: out projection =====================
        with tc.tile_pool(name="op_psum", bufs=2, space="PSUM") as opp, \
             tc.tile_pool(name="osb", bufs=3) as osp:
            for tt in range(NT):
                ps = opp.tile([128, DM], F32, tag="op")
                for nch in range(DM // 512):
                    sl = slice(nch * 512, (nch + 1) * 512)
                    for kt in range(OS // 128):
                        nc.tensor.matmul(
                            ps[:, sl],
                            lhsT=ygated_sb[:, kt, tt * 128:(tt + 1) * 128],
                            rhs=wo_sb[:, kt, sl],
                            start=(kt == 0), stop=False)
                    nc.tensor.matmul(ps[:, sl], lhsT=ones_sb[0:1],
                                     rhs=bo_sb[0:1, sl],
                                     start=False, stop=True)
                ot = osp.tile([128, DM], F32, tag="osb")
                nc.vector.tensor_copy(out=ot, in_=ps)
                nc.sync.dma_start(out=out[tt * 128:(tt + 1) * 128, :], in_=ot)


    nc.finalize()
    if split_waits:
        _split_waits(nc)
    return nc


_NC_CACHE = None


def _get_nc():
    global _NC_CACHE
    if _NC_CACHE is None:
        _NC_CACHE = build_nc()
    return _NC_CACHE


def _shard_inputs(query, key, value, Wq, bq, Wk, bk, Wv, bv, Wo, bo,
                  gate_w, gate_b):
    bf = ml_dtypes.bfloat16
    in_maps = []
    xT = {}
    for b in range(B):
        xT[b] = (np.ascontiguousarray(query[b].T).astype(bf),
                 np.ascontiguousarray(key[b].T).astype(bf),
                 np.ascontiguousarray(value[b].T).astype(bf))
    for c in range(8):
        b, hg = c // 2, c % 2
        sl = slice(hg * OS, (hg + 1) * OS)
        wg = np.zeros((D + 1, HL, D), np.float32)
        for hl in range(HL):
            h = hg * HL + hl
            wg[0:D, hl, :] = gate_w[h]
            wg[D, hl, :] = gate_b[h]
        in_maps.append({
            "xqT": xT[b][0], "xkT": xT[b][1], "xvT": xT[b][2],
            "wq": np.ascontiguousarray(Wq[:, sl]).astype(bf),
            "wk": np.ascontiguousarray(Wk[:, sl]).astype(bf),
            "wv": np.ascontiguousarray(Wv[:, sl]).astype(bf),
            "bq": np.ascontiguousarray(bq[sl].reshape(OS // 128, 128).T)
                    .astype(np.float32),
            "bk": np.ascontiguousarray(bk[sl].reshape(OS // 128, 128).T)
                    .astype(np.float32),
            "bv": bv[sl].reshape(1, OS).astype(bf),
            "wo": np.ascontiguousarray(Wo[sl, :]).astype(bf),
            "bo": (bo if hg == 0 else np.zeros_like(bo))
                    .reshape(1, DM).astype(bf),
            "wg": wg,
        })
    return in_maps


def kernel(query, key, value, Wq, bq, Wk, bk, Wv, bv, Wo, bo,
           gate_w, gate_b, _trace=False):
    nc = _get_nc()
    in_maps = _shard_inputs(
        np.asarray(query, np.float32), np.asarray(key, np.float32),
        np.asarray(value, np.float32),
        np.asarray(Wq, np.float32), np.asarray(bq, np.float32),
        np.asarray(Wk, np.float32), np.asarray(bk, np.float32),
        np.asarray(Wv, np.float32), np.asarray(bv, np.float32),
        np.asarray(Wo, np.float32), np.asarray(bo, np.float32),
        np.asarray(gate_w, np.float32), np.asarray(gate_b, np.float32))
    res = run_bass_kernel_spmd(nc, in_maps, core_ids=list(range(8)),
                               trace=_trace)
    out = np.empty((B, T, DM), np.float32)
    for b in range(B):
        out[b] = res.results[2 * b]["out"] + res.results[2 * b + 1]["out"]
    if _trace:
        kernel.last_exec_time_ns = res.exec_time_ns
        kernel.last_results = res
    return out
